# revision 8
# baseline (speedup 1.0000x reference)
"""Trainium2 Bass kernel for the sparse-attention (local 3x3 unfold) problem.

Semantics (per channel; the reference's .reshape is a RAW reinterpretation
of [9, L] -> [L, 9], so with RAW = patch-major unfold flattened [9*L]):
  out1_flat[n] = RAW_k[n] * RAW_q[9*(n//9) + 4]     (n in [0, 9L))
  out2_flat[n] = RAW_q[n] * RAW_k[9*(n//9) + 4]

Device design (v3, descriptor-size driven):
  * All device traffic is bf16 (harness gate is rel_err < 2e-2; bf16
    rounding of inputs + product is ~0.6% worst case).
  * The host pre-scrambles RAW into partition-major blobs: partition
    p = 16*chl + s of group g holds the contiguous flat span
    n in [9216*s, 9216*(s+1)) of channel g*8+chl.  A partition's RAW
    span is 72 distinct shifted image rows - incompressible - so loads
    are RAW-sized by necessity; making them partition-contiguous means
    every DMA (load and store) is [[9216,128],[1,9216]] <-> same-shape
    DRAM blob: 128 descriptors x 18,432 B per instruction, ~2k
    descriptors per core total (vs ~17k in the old baseline).
  * Compute is 2 vector multiplies per group (FD=9216): the center
    factor RAW[9*(n//9)+4] is a stride-9 read with 9x stride-0
    replication - one instruction covers a whole [128, 9216] tile.
  * Outputs are dumped as raw SBUF-ordered blobs and unscrambled +
    upcast on the host (flat n order == [L, 9] row-major, so the
    unscramble is a plain reshape).

Sharding: pure data-parallel over the 256 (b,c) channels; 32 per core.
Per-core traffic ~37.7 MB vs HBM-per-NC ~358 GB/s -> ~106 us floor.
"""

import sys

for _p in ("/opt/trn_rl_repo", "/opt/pypackages"):
    if _p not in sys.path:
        sys.path.insert(0, _p)

import numpy as np

import concourse.bass as bass
import concourse.mybir as mybir
import concourse.tile as tile
from concourse.bass import AP
from concourse.bass_utils import run_bass_kernel_spmd
from concourse.vector_clock import ScopedClock

# ---------------------------------------------------------------------------
# Patch: this container's walrus rejects >1 sync-wait on the Tile tail Drain
# ("Too many sync wait commands").  Spill extra waits onto SP NOPs, which
# execute in program order before the all-engine barrier, preserving the
# "all work done before sem clear" semantics.
# ---------------------------------------------------------------------------


def _drain_and_barrier(self, tick_clock, wait_clock):
    nc = self.nc
    drain_inst = nc.sync.drain()
    wait_clock.add_sem_waits(
        drain_inst.ins, ScopedClock({None: tick_clock.global_clock})
    )
    si = drain_inst.ins.sync_info
    if si is not None and len(si.on_wait) > 1:
        waits = list(si.on_wait)
        drain_inst.ins.sync_info = mybir.SyncInfo(
            on_wait=waits[:1], on_update=list(si.on_update)
        )
        for w in waits[1:]:
            nop = nc.sync.nop(nofuse=True)
            nop.ins.sync_info = mybir.SyncInfo(on_wait=[w], on_update=[])

    nc.all_engine_barrier()
    assert self.sems is not None
    popped = nc._tile_sem_poison_stack.pop()
    assert popped is self._sem_poison
    nc.clear_and_free_semaphores(list(self.sems.allocated().values()))
    nc.all_engine_barrier()


tile.TileContext._drain_and_barrier = _drain_and_barrier


def _split_waits(nc, maxw=1):
    """Walrus here allows only `maxw` sync-waits per instruction: move extra
    waits onto same-engine NOPs inserted immediately before the instruction
    (same engine stream => executes before it)."""
    for fn in nc.m.functions:
        for bb in fn.blocks:
            out = []
            for inst in bb.instructions:
                si = getattr(inst, "sync_info", None)
                if si is not None and len(si.on_wait) > maxw:
                    waits = list(si.on_wait)
                    for w in waits[:-maxw]:
                        nop = mybir.InstNoOp(
                            name=nc.get_next_instruction_name(),
                            bass_nofuse=True,
                        )
                        nop.engine = inst.engine
                        nop.sync_info = mybir.SyncInfo(on_wait=[w], on_update=[])
                        nc.register_instruction(nop)
                        out.append(nop)
                    inst.sync_info = mybir.SyncInfo(
                        on_wait=waits[-maxw:], on_update=list(si.on_update)
                    )
                out.append(inst)
            bb.instructions[:] = out

# ---------------------------------------------------------------------------

F32 = mybir.dt.float32
BF16 = mybir.dt.bfloat16
NP_BF16 = mybir.dt.np(mybir.dt.bfloat16)

N_CORES = 8
B, C, H, W = 4, 64, 128, 128
BC = B * C                # 256 channels
CPC = BC // N_CORES       # 32 channels per core
NG = 4                    # channel groups per core
NCH = CPC // NG           # 8 channels per group
SPC = 16                  # partitions per channel
XFREE = 9216              # flats per partition (= 9*L / 16, multiple of 9)
L = H * W


def _build_program():
    nc = bass.Bass(trn_type="TRN2")
    kp = nc.dram_tensor("kp", [NG, 128, XFREE], BF16, kind="ExternalInput")
    qp = nc.dram_tensor("qp", [NG, 128, XFREE], BF16, kind="ExternalInput")
    o1 = nc.dram_tensor("o1", [NG, 128, XFREE], BF16, kind="ExternalOutput")
    o2 = nc.dram_tensor("o2", [NG, 128, XFREE], BF16, kind="ExternalOutput")

    # DMA on the two HWDGE queues only (~30ns/descriptor each, ~285 GB/s
    # peak each); gpsimd (Pool) is reserved for offloaded multiplies --
    # a Q7 runs one instruction at a time, so SWDGE descriptor generation
    # would serialize behind 5us muls.
    engines = [nc.sync, nc.scalar]
    eng_i = [0]

    def eng():
        e = engines[eng_i[0] % len(engines)]
        eng_i[0] += 1
        return e

    NQ = 4                    # compute quarters per group
    XQ = XFREE // NQ          # 2304 flats per quarter (= 9*256)
    XH = XFREE // 2           # 4608 flats per half
    qdram = [[XFREE, 128], [1, XQ]]     # DRAM side, quarter
    qsb = [[XQ, 128], [1, XQ]]          # SBUF side, quarter tile
    hdram = [[XFREE, 128], [1, XH]]     # DRAM side, half
    hsb = [[XH, 128], [1, XH]]          # SBUF side, half tile
    qsb_out = [[XH, 128], [1, XQ]]      # quarter slice of a half tile
    with tile.TileContext(nc) as tc:
        with (
            tc.tile_pool(name="tin", bufs=3) as tin,
            tc.tile_pool(name="tin0", bufs=2) as tin0,
            tc.tile_pool(name="tout", bufs=4) as tout,
        ):
            for g in range(NG):
                o1t = o2t = tk = tq = None
                for q in range(NQ):
                    if q % 2 == 0:
                        o1t = tout.tile([128, XH], BF16, tag="o1t")
                        o2t = tout.tile([128, XH], BF16, tag="o2t")
                    # loads: quarter-grain for group 0 (fast pipeline
                    # start), half-grain for the rest (descriptor budget)
                    if g == 0:
                        tk = tin0.tile([128, XQ], BF16, tag="tk0")
                        tq = tin0.tile([128, XQ], BF16, tag="tq0")
                        for srcd, t in ((kp, tk), (qp, tq)):
                            eng().dma_start(
                                AP(t[:].tensor, 0, qsb),
                                AP(srcd, g * 128 * XFREE + q * XQ, qdram),
                            )
                        qo = 0
                        ipitch = XQ
                    else:
                        if q % 2 == 0:
                            tk = tin.tile([128, XH], BF16, tag="tk")
                            tq = tin.tile([128, XH], BF16, tag="tq")
                            for srcd, t in ((kp, tk), (qp, tq)):
                                eng().dma_start(
                                    AP(t[:].tensor, 0, hsb),
                                    AP(srcd, g * 128 * XFREE + (q // 2) * XH,
                                       hdram),
                                )
                        qo = (q % 2) * XQ
                        ipitch = XH

                    tkh, tqh = tk[:].tensor, tq[:].tensor
                    oo = (q % 2) * XQ
                    full = [[XH, 128], [9, XQ // 9], [1, 9]]
                    infull = [[ipitch, 128], [9, XQ // 9], [1, 9]]
                    bcast = [[ipitch, 128], [9, XQ // 9], [0, 9]]
                    nc.vector.tensor_mul(
                        AP(o1t[:].tensor, oo, full),
                        AP(tkh, qo, infull),
                        AP(tqh, qo + 4, bcast),
                    )
                    # odd quarters' out2 goes to gpsimd (~4.8us each);
                    # keeps the DVE chain ~61us, under the DMA window.
                    meng = nc.gpsimd if q % 2 == 1 else nc.vector
                    meng.tensor_mul(
                        AP(o2t[:].tensor, oo, full),
                        AP(tqh, qo, infull),
                        AP(tkh, qo + 4, bcast),
                    )

                    if q % 2 == 1:
                        doff = g * 128 * XFREE + (q // 2) * XH
                        if g == NG - 1:
                            # last group: quarter-grain stores (short tail)
                            for od, ot in ((o1, o1t), (o2, o2t)):
                                for sq in range(2):
                                    eng().dma_start(
                                        AP(od, doff + sq * XQ, qdram),
                                        AP(ot[:].tensor, sq * XQ, qsb_out),
                                    )
                        else:
                            for od, ot in ((o1, o1t), (o2, o2t)):
                                eng().dma_start(
                                    AP(od, doff, hdram),
                                    AP(ot[:].tensor, 0, hsb),
                                )
    _split_waits(nc)
    return nc


_NC_CACHE = []


def _get_nc():
    if not _NC_CACHE:
        _NC_CACHE.append(_build_program())
    return _NC_CACHE[0]


def _pretile(x):
    """[B,C,H,W] f32 -> [N_CORES, NG, 128, XFREE] bf16 RAW blobs.

    RAW[ch] = patch-major unfold [9, L] flattened; partition p=16*chl+s
    of group g holds RAW[ch=g*8+chl][9216*s : 9216*(s+1)]."""
    x = np.ascontiguousarray(np.asarray(x, dtype=np.float32).reshape(BC, H, W))
    pad = np.pad(x, ((0, 0), (1, 1), (1, 1)))          # [BC, 130, 130]
    raw = np.stack(
        [pad[:, di : di + H, dj : dj + W] for di in range(3) for dj in range(3)],
        axis=1,
    )                                                   # [BC, 9, H, W]
    return np.ascontiguousarray(
        raw.reshape(N_CORES, NG, 128, XFREE).astype(NP_BF16)
    )


def make_in_maps(key_map, query_map):
    kb = _pretile(key_map)
    qb = _pretile(query_map)
    return [{"kp": kb[m], "qp": qb[m]} for m in range(N_CORES)]


def assemble(results):
    outs = []
    for name in ("o1", "o2"):
        arr = np.stack([np.asarray(results[m][name]) for m in range(N_CORES)])
        # [core, g, p, f]: flat n = 9216*s + f per channel == [L, 9] raw order
        outs.append(arr.astype(np.float32).reshape(B, C, L, 9))
    return tuple(outs)


def kernel(key_map, query_map):
    nc = _get_nc()
    in_maps = make_in_maps(key_map, query_map)
    res = run_bass_kernel_spmd(nc, in_maps, core_ids=list(range(N_CORES)))
    return assemble(res.results)
